# revision 1
# baseline (speedup 1.0000x reference)
"""BatchConv2D (per-sample-weight conv) Trainium2 Bass kernel.

Problem: x [16,4,64,64,64], weight [16,128,64,3,3], bias [16,128] (all f32)
out[bi,bj] = conv2d(x[bi,bj], weight[bi], pad=1) + bias[bi]  -> [16,4,128,64,64]

Sharding: b_i axis split across 8 cores (2 per core); no communication.

Per-core kernel strategy ("parity-split dual-chain" conv-as-matmul, bf16):
  - Each image is stored in SBUF twice as [128 partitions, 33, 66]:
      img:     partitions 0..63 = EVEN padded rows, 64..127 = ODD padded rows
      imgSwap: the same data with the halves exchanged
    (parity slot r: even slot r = row 2r, odd slot r = row 2r-1; 1-pixel
    zero border baked in host-side). Both arrangements are pre-packed in
    DRAM host-side so each is ONE contiguous hardware-DGE DMA (a
    partition-crossed on-device swap costs ~11us/pass via the software-DGE
    path). The duplicate lets every conv tap of a given output-row parity
    be sourced from a single base partition:
      even outputs: ky=0/2 read img[64:], ky=1 reads imgSwap[64:]  (rows 64+)
      odd  outputs: ky=0/2 read img[:64], ky=1 reads imgSwap[:64]  (rows 0+)
  - Each 16-row output group therefore runs as TWO 9-matmul K=64 PSUM
    chains (one per parity) in opposite PE row-halves. Interleaving them
    1:1 keeps both halves streaming concurrently: 9 pair-slots x 512 rows
    = 4608 PE cycles per group — the 2-taps/cycle FLOP floor for a 3x3
    conv — while needing only 2 PSUM banks and 2 evictions per group.
  - Eviction splits across engines: DVE adds bias to the even-parity bank
    (TensorScalarPtr), the Activation engine does the odd bank
    (Identity activation with a per-partition bias AP). Both land in one
    per-image SBUF staging tile, stored by a single output DMA per image.
  - bf16 inputs AND outputs: 1 PE cycle/row, half the load bytes of fp32,
    half the store bytes (DMA time adds ~linearly to kernel time on this
    part); the host casts the output back to f32. Total rel-err ~2.8e-3
    against the f32 reference (harness gate 2e-2).
"""

import numpy as np

B_I, B_J, C, H, W = 16, 4, 64, 64, 64
OC, KH, KW = 128, 3, 3
N_CORES = 8
BPC = B_I // N_CORES          # b_i per core
NIMG = BPC * B_J              # images per core
RH = H // 2 + 1               # 33 rows per parity half (padded)
WP = W + 2                    # 66 padded width
GROUPS = 4                    # output row-groups of 16 rows per image
RG = H // GROUPS // 2         # 8 output rows per parity per group

_CACHE = {}


def _build_nc(repeat=1, hw_loop=None, bench_out=False, skip=()):
    """hw_loop/bench_out/skip are bench-only knobs (unused by kernel())."""
    import concourse.mybir as mybir
    from concourse import bacc, tile

    F32 = mybir.dt.float32
    BF16 = mybir.dt.float32r if "f32r" in skip else mybir.dt.bfloat16
    COPY = mybir.ActivationFunctionType.Identity

    xshape = [NIMG, 2, 2 * C, RH, WP]
    nc = bacc.Bacc("TRN2", target_bir_lowering=False, debug=False)
    if bench_out:
        x_d = nc.dram_tensor("x", xshape, BF16, kind="Internal")
        wt_d = nc.dram_tensor(
            "wt", [BPC, 2 * C, KH * KW, OC], BF16, kind="Internal"
        )
        b_d = nc.dram_tensor("bias", [OC, BPC], F32, kind="ExternalInput")
        o_d = nc.dram_tensor("oscr", [BPC, B_J, OC, H, W], BF16, kind="Internal")
        os_d = nc.dram_tensor("out", [OC, BPC], F32, kind="ExternalOutput")
    else:
        x_d = nc.dram_tensor("x", xshape, BF16, kind="ExternalInput")
        wt_d = nc.dram_tensor(
            "wt", [BPC, 2 * C, KH * KW, OC], BF16, kind="ExternalInput"
        )
        b_d = nc.dram_tensor("bias", [OC, BPC], F32, kind="ExternalInput")
        o_d = nc.dram_tensor("out", [BPC, B_J, OC, H, W], BF16, kind="ExternalOutput")

    with tile.TileContext(nc) as tc:
        with (
            tc.tile_pool(name="const", bufs=1) as cpool,
            tc.tile_pool(name="img", bufs=1) as ipool,
            tc.tile_pool(name="osb", bufs=1) as opool,
            tc.tile_pool(name="ps", bufs=1, space="PSUM") as pspool,
        ):
            wt_t = []
            for bi in range(BPC):
                w = cpool.tile([2 * C, KH * KW, OC], BF16, name=f"wt{bi}", tag=f"wt{bi}")
                nc.sync.dma_start(w[:, :, :], wt_d[bi])
                wt_t.append(w)
            bias_t = cpool.tile([OC, BPC], F32, name="bias_t", tag="bias")
            nc.sync.dma_start(bias_t[:, :], b_d[:, :])

            if "idma" in skip:
                img_c = cpool.tile([2 * C, RH, WP], BF16, name="imgc", tag="imgc")
                nc.sync.dma_start(img_c[:, :, :], x_d[0, 0])
                swp_c = cpool.tile([2 * C, RH, WP], BF16, name="swpc", tag="swpc")
                nc.sync.dma_start(swp_c[:, :, :], x_d[0, 1])

            if hw_loop is not None:
                loop_cm = tc.For_i(0, hw_loop, 1, name="rep")
                loop_cm.__enter__()

            for rep in range(repeat):
              for bi in range(BPC):
                for bj in range(B_J):
                    idx = bi * B_J + bj
                    ibufs = 8 if "bufs8" in skip else 4
                    if "idma" in skip:
                        img, swp = img_c, swp_c
                    else:
                        img = ipool.tile(
                            [2 * C, RH, WP], BF16, name="img", tag="img", bufs=ibufs
                        )
                        swp = ipool.tile(
                            [2 * C, RH, WP], BF16, name="swp", tag="swp", bufs=ibufs
                        )
                        if "ldsplit" in skip:
                            # per-half loads, half1 first (chain A's operands)
                            nc.sync.dma_start(img[C:, :, :], x_d[idx, 0, C:])
                            nc.scalar.dma_start(swp[C:, :, :], x_d[idx, 1, C:])
                            nc.sync.dma_start(img[0:C, :, :], x_d[idx, 0, 0:C])
                            nc.scalar.dma_start(swp[0:C, :, :], x_d[idx, 1, 0:C])
                        else:
                            nc.sync.dma_start(img[:, :, :], x_d[idx, 0])
                            swq = nc.gpsimd if "swpq" in skip else nc.scalar
                            swq.dma_start(swp[:, :, :], x_d[idx, 1])

                    osb = opool.tile(
                        [OC, GROUPS, RG, 2, W], BF16, name="osb", tag="osb",
                        bufs=3 if "osb3" in skip else 2,
                    )

                    for g in range(GROUPS):
                        r0 = g * RG
                        # Chain A: even output rows 2r, all rhs at base
                        # partition 64 (PE upper half). Chain C: odd rows
                        # 2r+1, all rhs at base 0 (lower half). 1:1
                        # interleave keeps both halves streaming.
                        psb = 4 if "psb4" in skip else 3
                        pst = {
                            k: pspool.tile(
                                [OC, RG, W], F32, name=f"ps{k}", tag=f"ps{k}",
                                bufs=psb,
                            )
                            for k in "AC"
                        }

                        def mm(bank, q, ky, kx, start, stop):
                            s = q + ky - 1
                            src = swp if ky == 1 else img
                            base = 64 * (1 - q)
                            rh0 = r0 + (1 if s >= 1 else 0)
                            nc.tensor.matmul(
                                pst[bank][:, :, :],
                                wt_t[bi][base : base + 64, KW * ky + kx, :],
                                src[
                                    base : base + 64,
                                    rh0 : rh0 + RG,
                                    kx : kx + W,
                                ],
                                start=start,
                                stop=stop,
                            )

                        kys = (0, 2, 1) if "tapord" in skip else (0, 1, 2)
                        taps = [(ky, kx) for ky in kys for kx in range(KW)]
                        for t, (ky, kx) in enumerate(taps):
                            mm("A", 0, ky, kx, start=(t == 0), stop=(t == 8))
                            mm("C", 1, ky, kx, start=(t == 0), stop=(t == 8))

                        if "dve" in skip:
                            continue
                        nc.vector.tensor_scalar_add(
                            osb[:, g, :, 0, :], pst["A"][:, :, :],
                            bias_t[:, bi : bi + 1],
                        )
                        nc.scalar.activation(
                            osb[:, g, :, 1, :], pst["C"][:, :, :],
                            COPY, bias=bias_t[:, bi : bi + 1],
                        )

                    if "dve" in skip or "odma" in skip:
                        continue
                    if "og4" in skip:
                        for g in range(GROUPS):
                            oq = nc.sync if g % 2 == 0 else nc.scalar
                            oq.dma_start(
                                o_d[bi, bj, :, 16 * g : 16 * g + 16, :],
                                osb[:, g, :, :, :],
                            )
                    else:
                        # SP queue: the Act queue already issues the swp
                        # loads and all evictions
                        nc.sync.dma_start(o_d[bi, bj], osb[:, :, :, :, :])

            if hw_loop is not None:
                loop_cm.__exit__(None, None, None)
            if bench_out:
                nc.sync.dma_start(os_d[:, :], bias_t[:, :])
    nc.compile()
    return nc


def _round_fp32r(a):
    """RNE-round fp32 to fp32r (11 stored mantissa bits; low 12 bits zero)."""
    u = a.view(np.uint32)
    r = (u + np.uint32(0x7FF) + ((u >> np.uint32(12)) & np.uint32(1))) & np.uint32(
        0xFFFFF000
    )
    return r.view(np.float32)


def _pack(x, weight, bias, f32r=False):
    """Host-side repack into the kernel's DMA-friendly layouts."""
    if f32r:
        bf16 = np.float32
        x = _round_fp32r(np.ascontiguousarray(x, dtype=np.float32))
        wq = _round_fp32r(np.ascontiguousarray(weight, dtype=np.float32))
    else:
        import ml_dtypes

        bf16 = ml_dtypes.bfloat16
        x = np.ascontiguousarray(x, dtype=np.float32).astype(bf16)
        wq = np.ascontiguousarray(weight, dtype=np.float32).astype(bf16)
    bias = np.ascontiguousarray(bias, dtype=np.float32)

    xp = np.zeros((B_I, B_J, 2, C, RH, WP), bf16)
    xp[:, :, 0, :, 0:32, 1 : W + 1] = x[:, :, :, 0::2, :]   # even slot r = row 2r
    xp[:, :, 1, :, 1:33, 1 : W + 1] = x[:, :, :, 1::2, :]   # odd slot r = row 2r-1
    xp = xp.reshape(B_I, B_J, 2 * C, RH, WP)
    # second copy with the partition halves pre-swapped (the swp-tile load)
    xp = np.stack(
        [xp, np.concatenate([xp[:, :, C:], xp[:, :, :C]], axis=2)], axis=2
    )

    wt0 = np.ascontiguousarray(np.transpose(wq, (0, 2, 3, 4, 1))).reshape(
        B_I, C, KH * KW, OC
    )
    wt = np.concatenate([wt0, wt0], axis=1)  # duplicate across partition halves

    bp = np.ascontiguousarray(np.transpose(bias, (1, 0)))  # [OC, B_I]
    return xp, wt, bp


def make_in_maps(xp, wt, bp):
    in_maps = []
    for i in range(N_CORES):
        sl = slice(i * BPC, (i + 1) * BPC)
        in_maps.append(
            {
                "x": np.ascontiguousarray(
                    xp[sl].reshape(NIMG, 2, 2 * C, RH, WP)
                ),
                "wt": np.ascontiguousarray(wt[sl]),
                "bias": np.ascontiguousarray(bp[:, sl]),
            }
        )
    return in_maps


def kernel(x, weight, bias):
    from concourse.bass_utils import run_bass_kernel_spmd

    xp, wt, bp = _pack(x, weight, bias)

    if "nc" not in _CACHE:
        _CACHE["nc"] = _build_nc()
    nc = _CACHE["nc"]

    in_maps = make_in_maps(xp, wt, bp)

    res = run_bass_kernel_spmd(nc, in_maps, list(range(N_CORES)))
    out = np.concatenate(
        [res.results[i]["out"].astype(np.float32) for i in range(N_CORES)], axis=0
    )
    return out

